# revision 20
# baseline (speedup 1.0000x reference)
"""Trainium2 Bass kernel for the ANFIS forward pass (8-core data-parallel).

Math: with L[b,f,m] = -0.5*((X[b,f]-mu[f,m])/sigma[f,m])^2,
  miAlloc[b,r] = prod_f exp(L[b,f,rules[r,f]])
  out[b] = (miAlloc @ c) / (sum_r miAlloc + 1e-10),  c = consequents.sum(1)

Factor the 8 features into two halves of 4. Each half has 81 possible
membership tuples, so miAlloc[b,r] = W1[b,rho1(r)] * W2[b,rho2(r)] where
  W1[b,t] = exp(sum_{f<4} a[f,tf]*(X[b,f]-mu[f,tf])^2),  a = -0.5/sigma^2
and rho1/rho2 map each rule to its half-tuple index. Then with
  C2[t1,t2] = sum_{r: rho(r)=(t1,t2)} c[r],   D2[t1,t2] = #{r: rho(r)=(t1,t2)}
(exact for arbitrary `rules`, duplicates included):
  num[b] = sum_{t2} (C2^T W1T)[t2,b] * W2T[t2,b]
  den[b] = sum_{t2} (D2^T W1T)[t2,b] * W2T[t2,b]
  out[b] = num[b] / (den[b] + 1e-10)

Per core (batch shard of 1024): one Square activation, two K=12 bf16
matmuls, exp over [81,1024], two K=81 bf16 matmuls, elementwise product,
ones-reduce matmul, then 1/(den+eps) via exp(-ln(den+eps)) on ScalarE.
"""

import numpy as np
import ml_dtypes

import concourse.bass as bass
import concourse.tile as tile
from concourse import bacc, mybir
from concourse.bass_utils import run_bass_kernel_spmd

B, F, M = 8192, 8, 3
NC = 8
BC = B // NC  # 1024 batch rows per core
T = M**4  # 81 tuples per feature-half
FP32 = mybir.dt.float32
BF16 = mybir.dt.bfloat16
AF = mybir.ActivationFunctionType
NP_BF16 = ml_dtypes.bfloat16

_CACHE = {}


def _build_graph():
    nc = bacc.Bacc("TRN2", target_bir_lowering=False, debug=False, num_devices=NC)

    # xt: col 0 = -mu (per-partition bias), cols 1.. = X^T replicated rows
    xt_ext = nc.dram_tensor("xt", [44, 1 + BC], FP32, kind="ExternalInput").ap()
    # bigc: [81, 275] bf16 = C2 | D2 | ones32 | eb (eb on rows 0:44)
    bigc_ext = nc.dram_tensor("bigc", [T, 3 * T + 32], BF16, kind="ExternalInput").ap()
    out_ext = nc.dram_tensor("out", [1, BC], FP32, kind="ExternalOutput").ap()

    with tile.TileContext(nc) as tc:
        with (
            tc.tile_pool(name="const", bufs=1) as const,
            tc.tile_pool(name="work", bufs=1) as work,
            tc.tile_pool(name="psum", bufs=1, space=bass.MemorySpace.PSUM) as psum,
        ):
            # only rows 0:12 and 32:44 carry data; split across the two HWDGE
            # queues so the transfer halves overlap
            xt = const.tile([44, 1 + BC], FP32)
            nc.vector.memset(xt[:, :], 0.0)
            nc.sync.dma_start(out=xt[0:12, :], in_=xt_ext[0:12, :])
            nc.scalar.dma_start(out=xt[32:44, :], in_=xt_ext[32:44, :])
            bigc = const.tile([T, 3 * T + 32], BF16)
            nc.gpsimd.dma_start(out=bigc[:, :], in_=bigc_ext[:, :])
            c2 = bigc[:, 0:T]
            d2 = bigc[:, T : 2 * T]
            ones32 = bigc[:, 2 * T : 2 * T + 32]
            eb_q0 = bigc[0:12, 2 * T + 32 : 3 * T + 32]
            eb_q32 = bigc[32:44, 2 * T + 32 : 3 * T + 32]
            epsb = const.tile([64, 1], FP32)
            nc.vector.memset(epsb[:, :], 1e-10)

            # PE warmup: dummy matmuls during the input-DMA window so the HAM
            # clock gate reaches 8/8 before the real matmuls issue
            wtile = const.tile([128, 512], BF16)
            nc.vector.memset(wtile[:, :], 0.0)
            warm = psum.tile([128, 512], FP32, tag="pa")
            for _ in range(10):
                nc.tensor.matmul(warm[:, :], lhsT=wtile[:, 0:128], rhs=wtile[:, :])

            # sq = (x - mu)^2, cast to bf16 for the matmul (rows 12:32 are
            # never read downstream; their garbage input is harmless)
            sq = work.tile([44, BC], BF16)
            nc.scalar.activation(
                sq[:, :], xt[:, 1 : 1 + BC], AF.Square, bias=xt[:, 0:1]
            )

            # logW: [81, 2048] = [logW1T halves | logW2T halves] (K=12, 2 row grps)
            lw = psum.tile([T, 2 * BC], FP32, tag="pa")
            for h in range(BC // 512):
                s = bass.ts(h, 512)
                nc.tensor.matmul(lw[:, s], lhsT=eb_q0, rhs=sq[0:12, s])
                nc.tensor.matmul(
                    lw[:, bass.ds(BC + h * 512, 512)], lhsT=eb_q32, rhs=sq[32:44, s]
                )

            w = work.tile([T, 2 * BC], BF16)
            nc.scalar.activation(w[:, :], lw[:, :], AF.Exp)
            w1 = w[:, 0:BC]
            w2 = w[:, BC : 2 * BC]

            # HT = C2^T @ W1T, HDT = D2^T @ W1T  (K=81)
            ht = psum.tile([T, BC], FP32, tag="pc")
            hd = psum.tile([T, BC], FP32, tag="pd")
            for h in range(BC // 512):
                s = bass.ts(h, 512)
                nc.tensor.matmul(ht[:, s], lhsT=c2, rhs=w1[:, s])
                nc.tensor.matmul(hd[:, s], lhsT=d2, rhs=w1[:, s])

            # products + partition-reduce, pipelined per 512-half; the reduce
            # lands halves on partitions 0 and 32 so the epilogue runs on two
            # lanes instead of one
            p1 = work.tile([T, BC], BF16)
            nc.vector.tensor_mul(p1[:, :], ht[:, :], w2)
            pd = work.tile([T, BC], BF16)
            nc.vector.tensor_mul(pd[:, :], hd[:, :], w2)

            # partition-reduce over t2; the [81,32] ones block broadcasts each
            # half's reduction onto a full 32-row block (rows 0:32 / 32:64) so
            # the epilogue reads only initialized partitions and runs on two
            # lanes (rows 0 and 32 are the ones DMA'd out)
            nps = psum.tile([64, 512], FP32, tag="pc")
            dps = psum.tile([64, 512], FP32, tag="pd")
            for h in range(BC // 512):
                s = bass.ts(h, 512)
                po = bass.ds(32 * h, 32)
                nc.tensor.matmul(nps[po, :], lhsT=ones32, rhs=p1[:, s])
                nc.tensor.matmul(dps[po, :], lhsT=ones32, rhs=pd[:, s])

            # out = num * 1/(den+eps); approx recip is ~51 ULP, plenty for 2e-2
            dene = work.tile([64, 512], FP32)
            nc.scalar.activation(dene[:, :], dps[:, :], AF.Identity, bias=epsb[:, :])
            rden = work.tile([64, 512], FP32)
            nc.vector.reciprocal_approx_fast(rden[:, :], dene[:, :])
            outt = work.tile([64, 512], FP32)
            nc.vector.tensor_mul(outt[:, :], nps[:, :], rden[:, :])

            nc.sync.dma_start(out=out_ext[:, 0:512], in_=outt[0:1, :])
            nc.scalar.dma_start(out=out_ext[:, 512:1024], in_=outt[32:33, :])

    nc.compile()
    return nc


def _get_graph():
    if "nc" not in _CACHE:
        _CACHE["nc"] = _build_graph()
    return _CACHE["nc"]


def _prep_inputs(X, mu, sigma, consequents, rules):
    X = np.ascontiguousarray(np.asarray(X, dtype=np.float32))
    mu = np.asarray(mu, dtype=np.float32)
    sigma = np.asarray(sigma, dtype=np.float32)
    c = np.asarray(consequents, dtype=np.float32).sum(axis=1)
    r = np.asarray(rules).astype(np.int64)

    a = (-0.5 / (np.asarray(sigma, np.float64) ** 2)).astype(np.float32)  # [F, M]

    # tuple digit f of t (m0 major), t in [0, 81)
    digits = (np.arange(T)[:, None] // np.array([27, 9, 3, 1])[None, :]) % 3  # [81, 4]

    eb = np.zeros((44, T), np.float32)
    negmu = np.zeros((44, 1), np.float32)
    for f in range(4):
        for m in range(3):
            sel = (digits[:, f] == m).astype(np.float32)
            eb[3 * f + m, :] = a[f, m] * sel
            eb[32 + 3 * f + m, :] = a[4 + f, m] * sel
            negmu[3 * f + m, 0] = -mu[f, m]
            negmu[32 + 3 * f + m, 0] = -mu[4 + f, m]

    Xsh = X.reshape(NC, BC, F)
    xt = np.zeros((NC, 44, 1 + BC), np.float32)
    xt[:, :, 0] = negmu[None, :, 0]
    for f in range(4):
        for m in range(3):
            xt[:, 3 * f + m, 1:] = Xsh[:, :, f]
            xt[:, 32 + 3 * f + m, 1:] = Xsh[:, :, 4 + f]

    rho1 = ((r[:, 0] * 3 + r[:, 1]) * 3 + r[:, 2]) * 3 + r[:, 3]
    rho2 = ((r[:, 4] * 3 + r[:, 5]) * 3 + r[:, 6]) * 3 + r[:, 7]
    C2 = np.zeros((T, T), np.float64)
    np.add.at(C2, (rho1, rho2), c.astype(np.float64))
    D2 = np.zeros((T, T), np.float64)
    np.add.at(D2, (rho1, rho2), 1.0)

    bigc = np.zeros((T, 3 * T + 32), np.float32)
    bigc[:, 0:T] = C2.astype(np.float32)
    bigc[:, T : 2 * T] = D2.astype(np.float32)
    bigc[:, 2 * T : 2 * T + 32] = 1.0
    bigc[0:44, 2 * T + 32 :] = eb
    bigc = np.ascontiguousarray(bigc.astype(NP_BF16))

    in_maps = [
        {"xt": np.ascontiguousarray(xt[i]), "bigc": bigc} for i in range(NC)
    ]
    return in_maps


def _run(in_maps, trace=False, **kwargs):
    nc = _get_graph()
    return run_bass_kernel_spmd(
        nc, in_maps, core_ids=list(range(NC)), trace=trace, **kwargs
    )


def kernel(X, mu, sigma, consequents, rules):
    in_maps = _prep_inputs(X, mu, sigma, consequents, rules)
    res = _run(in_maps)
    out = np.concatenate(
        [np.asarray(res.results[i]["out"]).reshape(BC) for i in range(NC)]
    )
    return out.astype(np.float32)


# revision 24
# speedup vs baseline: 1.0422x; 1.0422x over previous
"""Trainium2 Bass kernel for the ANFIS forward pass (8-core data-parallel).

Math: with L[b,f,m] = -0.5*((X[b,f]-mu[f,m])/sigma[f,m])^2,
  miAlloc[b,r] = prod_f exp(L[b,f,rules[r,f]])
  out[b] = (miAlloc @ c) / (sum_r miAlloc + 1e-10),  c = consequents.sum(1)

Factor the 8 features into two halves of 4. Each half has 81 possible
membership tuples, so miAlloc[b,r] = W1[b,rho1(r)] * W2[b,rho2(r)] where
  W1[b,t] = exp(sum_{f<4} a[f,tf]*(X[b,f]-mu[f,tf])^2),  a = -0.5/sigma^2
and rho1/rho2 map each rule to its half-tuple index. Then with
  C2[t1,t2] = sum_{r: rho(r)=(t1,t2)} c[r],   D2[t1,t2] = #{r: rho(r)=(t1,t2)}
(exact for arbitrary `rules`, duplicates included):
  num[b] = sum_{t2} (C2^T W1T)[t2,b] * W2T[t2,b]
  den[b] = sum_{t2} (D2^T W1T)[t2,b] * W2T[t2,b]
  out[b] = num[b] / (den[b] + 1e-10)

Per core (batch shard of 1024): one Square activation, two K=12 bf16
matmuls, exp over [81,1024], two K=81 bf16 matmuls, elementwise product,
ones-reduce matmul, then 1/(den+eps) via exp(-ln(den+eps)) on ScalarE.
"""

import numpy as np
import ml_dtypes

import concourse.bass as bass
import concourse.tile as tile
from concourse import bacc, mybir
from concourse.bass_utils import run_bass_kernel_spmd

B, F, M = 8192, 8, 3
NC = 8
BC = B // NC  # 1024 batch rows per core
T = M**4  # 81 tuples per feature-half
FP32 = mybir.dt.float32
BF16 = mybir.dt.bfloat16
AF = mybir.ActivationFunctionType
NP_BF16 = ml_dtypes.bfloat16

_CACHE = {}


def _build_graph():
    nc = bacc.Bacc("TRN2", target_bir_lowering=False, debug=False, num_devices=NC)

    # xt: col 0 = -mu (per-partition bias), cols 1.. = X^T replicated rows
    xt_ext = nc.dram_tensor("xt", [44, 1 + BC], FP32, kind="ExternalInput").ap()
    # bigc: [81, 275] bf16 = C2 | D2 | ones32 | eb (eb on rows 0:44)
    bigc_ext = nc.dram_tensor("bigc", [T, 3 * T + 32], BF16, kind="ExternalInput").ap()
    out_ext = nc.dram_tensor("out", [1, BC], FP32, kind="ExternalOutput").ap()

    with tile.TileContext(nc) as tc:
        with (
            tc.tile_pool(name="const", bufs=1) as const,
            tc.tile_pool(name="work", bufs=1) as work,
            tc.tile_pool(name="psum", bufs=1, space=bass.MemorySpace.PSUM) as psum,
        ):
            # PE warmup: dummy matmuls during the input-DMA window so the HAM
            # clock gate reaches 8/8 before the real matmuls issue
            wtile = const.tile([128, 512], BF16)
            nc.vector.memset(wtile[:, :], 0.0)
            warm = psum.tile([128, 512], FP32, tag="pa")
            for _ in range(7):
                nc.tensor.matmul(warm[:, :], lhsT=wtile[:, 0:128], rhs=wtile[:, :])

            # split the input across the two HWDGE queues so the halves overlap
            xt = const.tile([44, 1 + BC], FP32)
            nc.sync.dma_start(out=xt[0:22, :], in_=xt_ext[0:22, :])
            nc.scalar.dma_start(out=xt[22:44, :], in_=xt_ext[22:44, :])
            bigc = const.tile([T, 3 * T + 32], BF16)
            nc.gpsimd.dma_start(out=bigc[:, :], in_=bigc_ext[:, :])
            c2 = bigc[:, 0:T]
            d2 = bigc[:, T : 2 * T]
            ones32 = bigc[:, 2 * T : 2 * T + 32]
            eb_q0 = bigc[0:12, 2 * T + 32 : 3 * T + 32]
            eb_q32 = bigc[32:44, 2 * T + 32 : 3 * T + 32]
            epsb = const.tile([64, 1], FP32)
            nc.vector.memset(epsb[:, :], 1e-10)

            # sq = (x - mu)^2, cast to bf16 for the matmul (rows 12:32 are
            # never read downstream; their garbage input is harmless)
            sq = work.tile([44, BC], BF16)
            nc.scalar.activation(
                sq[:, :], xt[:, 1 : 1 + BC], AF.Square, bias=xt[:, 0:1]
            )

            # logW: [81, 2048] = [logW1T halves | logW2T halves] (K=12, 2 row grps)
            lw = psum.tile([T, 2 * BC], FP32, tag="pa")
            for h in range(BC // 512):
                s = bass.ts(h, 512)
                nc.tensor.matmul(lw[:, s], lhsT=eb_q0, rhs=sq[0:12, s])
                nc.tensor.matmul(
                    lw[:, bass.ds(BC + h * 512, 512)], lhsT=eb_q32, rhs=sq[32:44, s]
                )

            # mid-window dummies keep the PE busy while ScalarE runs the exps
            warm2 = psum.tile([128, 512], FP32, tag="pc")
            for _ in range(5):
                nc.tensor.matmul(warm2[:, :], lhsT=wtile[:, 0:128], rhs=wtile[:, :])

            # exp W1 first so the C2/D2 matmuls can start while W2's exp runs
            w = work.tile([T, 2 * BC], BF16)
            nc.scalar.activation(w[:, 0:BC], lw[:, 0:BC], AF.Exp)
            nc.scalar.activation(w[:, BC : 2 * BC], lw[:, BC : 2 * BC], AF.Exp)
            w1 = w[:, 0:BC]
            w2 = w[:, BC : 2 * BC]

            # HT = C2^T @ W1T, HDT = D2^T @ W1T  (K=81)
            ht = psum.tile([T, BC], FP32, tag="pc")
            hd = psum.tile([T, BC], FP32, tag="pd")
            for h in range(BC // 512):
                s = bass.ts(h, 512)
                nc.tensor.matmul(ht[:, s], lhsT=c2, rhs=w1[:, s])
            for h in range(BC // 512):
                s = bass.ts(h, 512)
                nc.tensor.matmul(hd[:, s], lhsT=d2, rhs=w1[:, s])

            # products + partition-reduce, pipelined per 512-half; the reduce
            # lands halves on partitions 0 and 32 so the epilogue runs on two
            # lanes instead of one
            p1 = work.tile([T, BC], BF16)
            nc.vector.tensor_mul(p1[:, :], ht[:, :], w2)
            pd = work.tile([T, BC], BF16)
            nc.vector.tensor_mul(pd[:, :], hd[:, :], w2)

            # partition-reduce over t2; the [81,32] ones block broadcasts each
            # half's reduction onto a full 32-row block (rows 0:32 / 32:64) so
            # the epilogue reads only initialized partitions and runs on two
            # lanes (rows 0 and 32 are the ones DMA'd out)
            nps = psum.tile([64, 512], FP32, tag="pc")
            dps = psum.tile([64, 512], FP32, tag="pd")
            for h in range(BC // 512):
                s = bass.ts(h, 512)
                po = bass.ds(32 * h, 32)
                nc.tensor.matmul(dps[po, :], lhsT=ones32, rhs=pd[:, s])
            for h in range(BC // 512):
                s = bass.ts(h, 512)
                po = bass.ds(32 * h, 32)
                nc.tensor.matmul(nps[po, :], lhsT=ones32, rhs=p1[:, s])

            # out = num * 1/(den+eps); approx recip is ~51 ULP, plenty for 2e-2
            dene = work.tile([64, 512], FP32)
            nc.scalar.activation(dene[:, :], dps[:, :], AF.Identity, bias=epsb[:, :])
            rden = work.tile([64, 512], FP32)
            nc.vector.reciprocal_approx_fast(rden[:, :], dene[:, :])
            outt = work.tile([64, 512], FP32)
            nc.vector.tensor_mul(outt[:, :], nps[:, :], rden[:, :])

            nc.sync.dma_start(out=out_ext[:, 0:512], in_=outt[0:1, :])
            nc.scalar.dma_start(out=out_ext[:, 512:1024], in_=outt[32:33, :])

    nc.compile()
    return nc


def _get_graph():
    if "nc" not in _CACHE:
        _CACHE["nc"] = _build_graph()
    return _CACHE["nc"]


def _prep_inputs(X, mu, sigma, consequents, rules):
    X = np.ascontiguousarray(np.asarray(X, dtype=np.float32))
    mu = np.asarray(mu, dtype=np.float32)
    sigma = np.asarray(sigma, dtype=np.float32)
    c = np.asarray(consequents, dtype=np.float32).sum(axis=1)
    r = np.asarray(rules).astype(np.int64)

    a = (-0.5 / (np.asarray(sigma, np.float64) ** 2)).astype(np.float32)  # [F, M]

    # tuple digit f of t (m0 major), t in [0, 81)
    digits = (np.arange(T)[:, None] // np.array([27, 9, 3, 1])[None, :]) % 3  # [81, 4]

    eb = np.zeros((44, T), np.float32)
    negmu = np.zeros((44, 1), np.float32)
    for f in range(4):
        for m in range(3):
            sel = (digits[:, f] == m).astype(np.float32)
            eb[3 * f + m, :] = a[f, m] * sel
            eb[32 + 3 * f + m, :] = a[4 + f, m] * sel
            negmu[3 * f + m, 0] = -mu[f, m]
            negmu[32 + 3 * f + m, 0] = -mu[4 + f, m]

    Xsh = X.reshape(NC, BC, F)
    xt = np.zeros((NC, 44, 1 + BC), np.float32)
    xt[:, :, 0] = negmu[None, :, 0]
    for f in range(4):
        for m in range(3):
            xt[:, 3 * f + m, 1:] = Xsh[:, :, f]
            xt[:, 32 + 3 * f + m, 1:] = Xsh[:, :, 4 + f]

    rho1 = ((r[:, 0] * 3 + r[:, 1]) * 3 + r[:, 2]) * 3 + r[:, 3]
    rho2 = ((r[:, 4] * 3 + r[:, 5]) * 3 + r[:, 6]) * 3 + r[:, 7]
    C2 = np.zeros((T, T), np.float64)
    np.add.at(C2, (rho1, rho2), c.astype(np.float64))
    D2 = np.zeros((T, T), np.float64)
    np.add.at(D2, (rho1, rho2), 1.0)

    bigc = np.zeros((T, 3 * T + 32), np.float32)
    bigc[:, 0:T] = C2.astype(np.float32)
    bigc[:, T : 2 * T] = D2.astype(np.float32)
    bigc[:, 2 * T : 2 * T + 32] = 1.0
    bigc[0:44, 2 * T + 32 :] = eb
    bigc = np.ascontiguousarray(bigc.astype(NP_BF16))

    in_maps = [
        {"xt": np.ascontiguousarray(xt[i]), "bigc": bigc} for i in range(NC)
    ]
    return in_maps


def _run(in_maps, trace=False, **kwargs):
    nc = _get_graph()
    return run_bass_kernel_spmd(
        nc, in_maps, core_ids=list(range(NC)), trace=trace, **kwargs
    )


def kernel(X, mu, sigma, consequents, rules):
    in_maps = _prep_inputs(X, mu, sigma, consequents, rules)
    res = _run(in_maps)
    out = np.concatenate(
        [np.asarray(res.results[i]["out"]).reshape(BC) for i in range(NC)]
    )
    return out.astype(np.float32)
